# revision 3
# baseline (speedup 1.0000x reference)
"""Autoregressive GRU on 8 TRN2 NeuronCores — pair-sharded v5\n(= v2 with the PE matmuls phase-split: own-tile half first, partner-gated\nhalf second, so the exchange latency hides under real work).

Problem: B=256, D=1024, T=128 decode steps.
  step:  z = sig(inp@Wz + h@Uz + bz); r = sig(inp@Wr + h@Ur + br)
         hh = tanh(inp@Wh + bh + r*(h@Uh));  h' = z*h + (1-z)*hh
  inp(0) = 0, h(0) = x, inp(t) == h(t) for t >= 1, so steps >= 1 use the
  fused Gz = Wz+Uz, Gr = Wr+Ur plus Wh, Uh ("xh"/"hl") separately.

Sharding: 4 batch quarters x 2 feature halves. Core c owns batch rows
[(c>>1)*64, +64) and feature half m = c&1 ([m*512, +512)). The pair
(c, c^1) shares a batch quarter; each step a core computes its 512
features for its 64 batch rows and exchanges the transposed fp16 state
chunk ([128, 256], 64 KB) with its XOR-1 partner — ONE remote send per
step per core. (The v1 layout needed 7 sends/step, and 7 outstanding
SWDGE preps cost ~42 us/step on the Q7; 1 prep + 1 trigger is ~2.7 us.)

Matmuls are batch-major: state tile hT[k] [128 feat, 64 batch] is
stationary, weight blocks stream as rhs [128, 512], accumulating
out[batch, feat] in PSUM. z/r share PSUM bank A (z on partitions 0:64,
r on 64:128 via the out-AP partition offset) so the sigmoid is a single
[128, 512] ACT op; hl and xh get their own banks (PE-write + DVE-read of
the same bank is a hard fault). Weight loads hide under the 512-cycle
streams (measured: matmul cost ~= stream cycles only, LDW is pulled
ahead by the PE's reorder window).

The new state [64 batch, 512 feat] fp16 is PE-transposed back to four
[128, 64] tiles for the next step's stationaries. 128 steps fully
unrolled, cross-engine order by explicit semaphores.
"""

import numpy as np

B = 256          # batch
D = 1024         # hidden
T = 128          # decode steps
NCORES = 8
BQ = B // 4      # batch rows per core = 64
FH = D // 2      # features per core = 512
KT = 8           # contraction k-tiles of 128


def _build(t_steps: int, with_bias: bool):
    import concourse.bass as bass
    import concourse.mybir as mybir
    from concourse import bacc

    f16 = mybir.dt.float16
    f32 = mybir.dt.float32
    Alu = mybir.AluOpType
    Act = mybir.ActivationFunctionType

    nc = bacc.Bacc()

    # ---- external I/O (per core) ----
    # wg: steps>=1 weights fp16. Block tt at cols tt*2048, layout
    #     [z(512) | r(512) | xh(512) | hl(512)]; rows = k-tile tt's global
    #     feature rows (tt 0-3 own half, 4-7 partner half).
    wg = nc.declare_dram_parameter("wg", [128, KT * 2048], f16, isOutput=False)
    # u1: step-0 weights [Uz(512) | Ur(512) | Uh(512)] per k-tile block.
    u1 = nc.declare_dram_parameter("u1", [128, KT * 1536], f16, isOutput=False)
    # ht0: initial transposed state fp16: 8 tiles [128 feat, 64 batch]
    ht0 = nc.declare_dram_parameter("ht0", [128, 512], f16, isOutput=False)
    # x32: initial own h slice fp32 [64 batch, 512 feat]
    x32 = nc.declare_dram_parameter("x32", [64, 512], f32, isOutput=False)
    ident = nc.declare_dram_parameter("ident", [64, 64], f16, isOutput=False)
    if with_bias:
        bias_a = nc.declare_dram_parameter("bias_a", [128, 512], f32, isOutput=False)
        bias_h = nc.declare_dram_parameter("bias_h", [64, 512], f32, isOutput=False)
    out = nc.declare_dram_parameter("out", [t_steps, 64, 512], f32, isOutput=True)

    # ---- SBUF ----
    wg_sb = nc.alloc_sbuf_tensor("wg_sb", [128, KT * 2048], f16)
    u1_sb = nc.alloc_sbuf_tensor("u1_sb", [128, KT * 1536], f16)
    # ht[p]: slot0 = own 4 tiles (cols 0:256), slot1 = partner (256:512)
    ht_sb = [nc.alloc_sbuf_tensor(f"ht{p}_sb", [128, 512], f16) for p in (0, 1)]
    h32_sb = [nc.alloc_sbuf_tensor(f"h32{p}_sb", [64, 512], f32) for p in (0, 1)]
    zr_sb = nc.alloc_sbuf_tensor("zr_sb", [128, 512], f32)  # z rows 0:64, r 64:128
    t1_sb = nc.alloc_sbuf_tensor("t1_sb", [64, 512], f32)
    t2_sb = nc.alloc_sbuf_tensor("t2_sb", [64, 512], f32)
    hh_sb = nc.alloc_sbuf_tensor("hh_sb", [64, 512], f32)
    d_sb = nc.alloc_sbuf_tensor("d_sb", [64, 512], f32)
    m_sb = nc.alloc_sbuf_tensor("m_sb", [64, 512], f32)
    h16_sb = nc.alloc_sbuf_tensor("h16_sb", [64, 512], f16)
    ident_sb = nc.alloc_sbuf_tensor("ident_sb", [64, 64], f16)
    if with_bias:
        zrb_sb = nc.alloc_sbuf_tensor("zrb_sb", [128, 512], f32)
        bias_a_sb = nc.alloc_sbuf_tensor("bias_a_sb", [128, 512], f32)
        bias_h_sb = nc.alloc_sbuf_tensor("bias_h_sb", [64, 512], f32)

    # ---- PSUM (bank-disjoint readers/writers) ----
    psA = [nc.alloc_psum_tensor(f"psA{p}", [128, 512], f32) for p in (0, 1)]
    psH = [nc.alloc_psum_tensor(f"psH{p}", [64, 512], f32) for p in (0, 1)]  # hl
    psX = [nc.alloc_psum_tensor(f"psX{p}", [64, 512], f32) for p in (0, 1)]  # xh
    psT = [nc.alloc_psum_tensor(f"psT{p}", [128, 256], f16) for p in (0, 1)]

    # ---- semaphores ----
    init_sem = nc.alloc_semaphore("init_sem")
    mm_sem = nc.alloc_semaphore("mm_sem")    # +4/step: A, hl, xh, transposes
    act_sem = nc.alloc_semaphore("act_sem")  # +3/step: sigmoid, tanh, slot0 copy
    dve_sem = nc.alloc_semaphore("dve_sem")  # +3/step: tanh-in, h16, h32
    rsem1 = nc.alloc_semaphore("rsem1")      # partner S1 arrivals: +2/step
    bsem = nc.alloc_semaphore("bsem")        # local send complete: +16/step
    prep_sem = nc.alloc_semaphore("prep_sem")
    out_sem = nc.alloc_semaphore("out_sem")
    if with_bias:
        zb_sem = nc.alloc_semaphore("zb_sem")

    N_LOADS = (7 if with_bias else 5)

    def htile(p, tt):
        return ht_sb[p][:, tt * 64:(tt + 1) * 64]

    with nc.Block() as block:

        @block.sync
        def _(sync):
            sync.dma_start(out=wg_sb[:, :], in_=wg[:, :]).then_inc(init_sem, 16)
            sync.dma_start(out=u1_sb[:, :], in_=u1[:, :]).then_inc(init_sem, 16)
            sync.dma_start(out=ht_sb[0][:, :], in_=ht0[:, :]).then_inc(init_sem, 16)
            sync.dma_start(out=h32_sb[0][:, :], in_=x32[:, :]).then_inc(init_sem, 16)
            sync.dma_start(out=ident_sb[:, :], in_=ident[:, :]).then_inc(init_sem, 16)
            if with_bias:
                sync.dma_start(out=bias_a_sb[:, :], in_=bias_a[:, :]).then_inc(init_sem, 16)
                sync.dma_start(out=bias_h_sb[:, :], in_=bias_h[:, :]).then_inc(init_sem, 16)
            for t in range(t_steps):
                nxt = (t + 1) % 2
                sync.dma_start(out=out[t], in_=h32_sb[nxt][:, :]).then_inc(
                    out_sem, 16)._wait_ge(dve_sem, 3 * t + 3)

        @block.tensor
        def _(tensor):
            for t in range(t_steps):
                par = t % 2
                rhs_w, blk = (u1_sb, 1536) if t == 0 else (wg_sb, 2048)
                # passes: (psum-ap-maker, col-offset-in-block, start, inc)
                if t == 0:
                    # u1 layout [Uz | Ur | Uh]; no xh pass, hl tail incs +2
                    passes = [
                        (psA[par][0:64, :], 0, True, 0),        # z
                        (psA[par][64:128, :], 512, True, 1),    # r -> A done
                        (psH[par][:, :], 1024, True, 2),        # hl (+2: no xh)
                    ]
                else:
                    passes = [
                        (psA[par][0:64, :], 0, True, 0),        # z
                        (psA[par][64:128, :], 512, True, 1),    # r -> A done
                        (psH[par][:, :], 1536, True, 1),        # hl
                        (psX[par][:, :], 1024, True, 1),        # xh
                    ]
                # phase A: all own-tile (slot0) matmuls of every pass, so
                # the PE never stalls on the partner exchange until half the
                # step's matmuls are already issued; phase B (slot1 tiles)
                # carries the rsem1 gate once. Accumulation groups interleave
                # across banks (start on first phase-A touch, stop on last
                # phase-B touch).
                for phase, tts in ((0, range(0, KT // 2)),
                                   (1, range(KT // 2, KT))):
                    for pi, (ps_ap, co, start0, inc) in enumerate(passes):
                        for tt in tts:
                            mm = tensor.matmul(
                                ps_ap,
                                htile(par, tt),
                                rhs_w[:, tt * blk + co: tt * blk + co + 512],
                                start=(tt == 0 and start0),
                                stop=(tt == KT - 1),
                                skip_group_check=True)
                            if phase == 0 and pi == 0 and tt == 0:
                                if t == 0:
                                    mm._wait_ge(init_sem, 16 * N_LOADS)
                                else:
                                    # own slot0 tiles: ACT copy of step t-1
                                    mm._wait_ge(act_sem, 3 * t)
                            if phase == 1 and pi == 0 and tt == KT // 2 \
                                    and t > 0:
                                # partner slot1 tiles: S1 of step t-1 landed
                                mm._wait_ge(rsem1, 2 * t)
                        if phase == 1 and inc:
                            mm.then_inc(mm_sem, inc)
                # transposes: h16 [64, 512] -> four [128, 64] tiles
                for i in range(4):
                    tr = tensor.transpose(
                        psT[par][:, i * 64:(i + 1) * 64],
                        h16_sb[:, i * 128:(i + 1) * 128],
                        ident_sb[:, :])
                    if i == 0:
                        tr._wait_ge(dve_sem, 3 * t + 2)
                tr.then_inc(mm_sem, 1)

        @block.scalar
        def _(scalar):
            for t in range(t_steps):
                par, nxt = t % 2, (t + 1) % 2
                if with_bias:
                    sig = scalar.activation(zr_sb[:, :], zrb_sb[:, :],
                                            Act.Sigmoid)._wait_ge(zb_sem, t + 1)
                else:
                    sig = scalar.activation(zr_sb[:, :], psA[par][:, :],
                                            Act.Sigmoid)._wait_ge(mm_sem, 4 * t + 1)
                sig.then_inc(act_sem, 1)
                tin = t1_sb if t == 0 else t2_sb
                scalar.activation(hh_sb[:, :], tin[:, :], Act.Tanh)._wait_ge(
                    dve_sem, 3 * t + 1).then_inc(act_sem, 1)
                scalar.copy(ht_sb[nxt][:, 0:256], psT[par][:, :])._wait_ge(
                    mm_sem, 4 * t + 4).then_inc(act_sem, 1)

        @block.vector
        def _(vector):
            for t in range(t_steps):
                par, nxt = t % 2, (t + 1) % 2
                if with_bias:
                    # zrb = psA + bias (sigmoid input); done on DVE since the
                    # ACT bias operand is per-partition and batch-major bias
                    # varies along the free dim. b==0 compiles this out.
                    vector.tensor_tensor(zrb_sb[:, :], psA[par][:, :],
                                         bias_a_sb[:, :], Alu.add)._wait_ge(
                        mm_sem, 4 * t + 1).then_inc(zb_sem, 1)
                # t1 = r * hl (needs sigmoid AND the hl pass; one wait per
                # instruction, so the act wait is standalone)
                vector.wait_ge(act_sem, 3 * t + 1)
                tt1 = vector.tensor_tensor(
                    t1_sb[:, :], zr_sb[64:128, :], psH[par][:, :],
                    Alu.mult)._wait_ge(mm_sem, 4 * t + 2)
                if t == 0:
                    last = tt1
                    if with_bias:
                        last = vector.tensor_tensor(t1_sb[:, :], t1_sb[:, :],
                                                    bias_h_sb[:, :], Alu.add)
                else:
                    last = vector.tensor_tensor(
                        t2_sb[:, :], t1_sb[:, :], psX[par][:, :],
                        Alu.add)._wait_ge(mm_sem, 4 * t + 3)
                    if with_bias:
                        last = vector.tensor_tensor(t2_sb[:, :], t2_sb[:, :],
                                                    bias_h_sb[:, :], Alu.add)
                last.then_inc(dve_sem, 1)
                # d = h - hh ; m = z * d ; h' = hh + m
                vector.tensor_tensor(d_sb[:, :], h32_sb[par][:, :], hh_sb[:, :],
                                     Alu.subtract)._wait_ge(act_sem, 3 * t + 2)
                vector.tensor_tensor(m_sb[:, :], zr_sb[0:64, :], d_sb[:, :],
                                     Alu.mult)
                vector.tensor_tensor(h16_sb[:, :], hh_sb[:, :], m_sb[:, :],
                                     Alu.add).then_inc(dve_sem, 1)
                if t >= 2:
                    vector.wait_ge(out_sem, 16 * (t - 1))
                vector.tensor_tensor(h32_sb[nxt][:, :], hh_sb[:, :], m_sb[:, :],
                                     Alu.add).then_inc(dve_sem, 1)

        @block.gpsimd
        def _(gpsimd):
            for t in range(t_steps - 1):
                nxt = (t + 1) % 2
                rdests = [None] * NCORES
                rdests[1] = (0, 1)
                gpsimd.remote_dma_broadcast(
                    ht_sb[nxt][:, 256:512],
                    ht_sb[nxt][:, 0:256],
                    remote_sem=rsem1,
                    local_sem=bsem,
                    rdests=rdests,
                ).then_inc(prep_sem, 1)
                gpsimd.trigger_dma(1)._wait_ge(act_sem, 3 * t + 3)

    nc.compile()
    return nc


# ---------------------------------------------------------------------------
# host side
# ---------------------------------------------------------------------------

def _prep_inputs(x, W, U, b):
    x = np.asarray(x, np.float32)
    W = np.asarray(W, np.float32)
    U = np.asarray(U, np.float32)
    b = np.asarray(b, np.float32)
    with_bias = bool(np.any(b != 0.0))

    Wz, Wr, Wh = W[:, :D], W[:, D:2 * D], W[:, 2 * D:]
    Uz, Ur, Uh = U[:, :D], U[:, D:2 * D], U[:, 2 * D:]
    G = [Wz + Uz, Wr + Ur, Wh, Uh]   # z | r | xh | hl
    U1 = [Uz, Ur, Uh]

    in_maps = []
    for c in range(NCORES):
        m, p = c & 1, c >> 1
        bsl = slice(p * BQ, (p + 1) * BQ)
        fsl = slice(m * FH, (m + 1) * FH)
        wblocks, ublocks, hblocks = [], [], []
        for j in (0, 1):             # slot: 0 = own half, 1 = partner half
            half = m ^ j
            for i in range(4):       # k-tile within the half
                rows = slice(half * FH + i * 128, half * FH + (i + 1) * 128)
                wblocks.append(np.hstack([g[rows, fsl] for g in G]))
                ublocks.append(np.hstack([g[rows, fsl] for g in U1]))
                hblocks.append(x[bsl, rows].T)
        in_map = {
            "wg": np.ascontiguousarray(np.hstack(wblocks)).astype(np.float16),
            "u1": np.ascontiguousarray(np.hstack(ublocks)).astype(np.float16),
            "ht0": np.ascontiguousarray(np.hstack(hblocks)).astype(np.float16),
            "x32": np.ascontiguousarray(x[bsl, fsl]),
            "ident": np.eye(64, dtype=np.float16),
        }
        if with_bias:
            bz, br, bh = b[:D][fsl], b[D:2 * D][fsl], b[2 * D:][fsl]
            in_map["bias_a"] = np.ascontiguousarray(
                np.vstack([np.tile(bz, (64, 1)), np.tile(br, (64, 1))])
            ).astype(np.float32)
            in_map["bias_h"] = np.ascontiguousarray(
                np.tile(bh, (64, 1))).astype(np.float32)
        in_maps.append(in_map)
    return in_maps, with_bias


def gather(results, t_steps=T):
    """results: per-core dicts with 'out' [t_steps, 64, 512] -> [B, T, D]."""
    full = np.empty((B, t_steps, D), np.float32)
    for c in range(NCORES):
        m, p = c & 1, c >> 1
        co = np.asarray(results[c]["out"]).reshape(t_steps, BQ, FH)
        full[p * BQ:(p + 1) * BQ, :, m * FH:(m + 1) * FH] = \
            np.transpose(co, (1, 0, 2))
    return full


def run(x, W, U, b, trace=False, t_steps=T, **spmd_kwargs):
    import sys
    if "/opt/trn_rl_repo" not in sys.path:
        sys.path.insert(0, "/opt/trn_rl_repo")
    from concourse.bass_utils import run_bass_kernel_spmd

    in_maps, with_bias = _prep_inputs(x, W, U, b)
    nc = _build(t_steps, with_bias)
    res = run_bass_kernel_spmd(nc, in_maps, core_ids=list(range(NCORES)),
                               trace=trace, **spmd_kwargs)
    full = np.empty((B, t_steps, D), np.float32)
    for c in range(NCORES):
        m, p = c & 1, c >> 1
        co = np.asarray(res.results[c]["out"])  # [T, 64, 512]
        full[p * BQ:(p + 1) * BQ, :, m * FH:(m + 1) * FH] = \
            np.transpose(co, (1, 0, 2))
    return full, res


def kernel(x, W, U, b):
    return run(x, W, U, b)[0]


# revision 4
# speedup vs baseline: 1.0016x; 1.0016x over previous
"""Autoregressive GRU on 8 TRN2 NeuronCores — pair-sharded v5\n(= v2 with the PE matmuls phase-split: own-tile half first, partner-gated\nhalf second, so the exchange latency hides under real work).

Problem: B=256, D=1024, T=128 decode steps.
  step:  z = sig(inp@Wz + h@Uz + bz); r = sig(inp@Wr + h@Ur + br)
         hh = tanh(inp@Wh + bh + r*(h@Uh));  h' = z*h + (1-z)*hh
  inp(0) = 0, h(0) = x, inp(t) == h(t) for t >= 1, so steps >= 1 use the
  fused Gz = Wz+Uz, Gr = Wr+Ur plus Wh, Uh ("xh"/"hl") separately.

Sharding: 4 batch quarters x 2 feature halves. Core c owns batch rows
[(c>>1)*64, +64) and feature half m = c&1 ([m*512, +512)). The pair
(c, c^1) shares a batch quarter; each step a core computes its 512
features for its 64 batch rows and exchanges the transposed fp16 state
chunk ([128, 256], 64 KB) with its XOR-1 partner — ONE remote send per
step per core. (The v1 layout needed 7 sends/step, and 7 outstanding
SWDGE preps cost ~42 us/step on the Q7; 1 prep + 1 trigger is ~2.7 us.)

Matmuls are batch-major: state tile hT[k] [128 feat, 64 batch] is
stationary, weight blocks stream as rhs [128, 512], accumulating
out[batch, feat] in PSUM. z/r share PSUM bank A (z on partitions 0:64,
r on 64:128 via the out-AP partition offset) so the sigmoid is a single
[128, 512] ACT op; hl and xh get their own banks (PE-write + DVE-read of
the same bank is a hard fault). Weight loads hide under the 512-cycle
streams (measured: matmul cost ~= stream cycles only, LDW is pulled
ahead by the PE's reorder window).

The new state [64 batch, 512 feat] fp16 is PE-transposed back to four
[128, 64] tiles for the next step's stationaries. 128 steps fully
unrolled, cross-engine order by explicit semaphores.
"""

import numpy as np

B = 256          # batch
D = 1024         # hidden
T = 128          # decode steps
NCORES = 8
BQ = B // 4      # batch rows per core = 64
FH = D // 2      # features per core = 512
KT = 8           # contraction k-tiles of 128


def _build(t_steps: int, with_bias: bool):
    import concourse.bass as bass
    import concourse.mybir as mybir
    from concourse import bacc

    f16 = mybir.dt.float16
    f32 = mybir.dt.float32
    Alu = mybir.AluOpType
    Act = mybir.ActivationFunctionType

    nc = bacc.Bacc()

    # ---- external I/O (per core) ----
    # wg: steps>=1 weights fp16. Block tt at cols tt*2048, layout
    #     [z(512) | r(512) | xh(512) | hl(512)]; rows = k-tile tt's global
    #     feature rows (tt 0-3 own half, 4-7 partner half).
    wg = nc.declare_dram_parameter("wg", [128, KT * 2048], f16, isOutput=False)
    # u1: step-0 weights [Uz(512) | Ur(512) | Uh(512)] per k-tile block.
    u1 = nc.declare_dram_parameter("u1", [128, KT * 1536], f16, isOutput=False)
    # ht0: initial transposed state fp16: 8 tiles [128 feat, 64 batch]
    ht0 = nc.declare_dram_parameter("ht0", [128, 512], f16, isOutput=False)
    # x32: initial own h slice fp32 [64 batch, 512 feat]
    x32 = nc.declare_dram_parameter("x32", [64, 512], f32, isOutput=False)
    ident = nc.declare_dram_parameter("ident", [64, 64], f32, isOutput=False)
    if with_bias:
        bias_a = nc.declare_dram_parameter("bias_a", [128, 512], f32, isOutput=False)
        bias_h = nc.declare_dram_parameter("bias_h", [64, 512], f32, isOutput=False)
    out = nc.declare_dram_parameter("out", [t_steps, 64, 512], f32, isOutput=True)

    # ---- SBUF ----
    wg_sb = nc.alloc_sbuf_tensor("wg_sb", [128, KT * 2048], f16)
    u1_sb = nc.alloc_sbuf_tensor("u1_sb", [128, KT * 1536], f16)
    # ht[p]: slot0 = own 4 tiles (cols 0:256), slot1 = partner (256:512)
    ht_sb = [nc.alloc_sbuf_tensor(f"ht{p}_sb", [128, 512], f16) for p in (0, 1)]
    h32_sb = [nc.alloc_sbuf_tensor(f"h32{p}_sb", [64, 512], f32) for p in (0, 1)]
    zr_sb = nc.alloc_sbuf_tensor("zr_sb", [128, 512], f32)  # z rows 0:64, r 64:128
    t1_sb = nc.alloc_sbuf_tensor("t1_sb", [64, 512], f32)
    t2_sb = nc.alloc_sbuf_tensor("t2_sb", [64, 512], f32)
    hh_sb = nc.alloc_sbuf_tensor("hh_sb", [64, 512], f32)
    d_sb = nc.alloc_sbuf_tensor("d_sb", [64, 512], f32)
    m_sb = nc.alloc_sbuf_tensor("m_sb", [64, 512], f32)
    ident_sb = nc.alloc_sbuf_tensor("ident_sb", [64, 64], f32)
    if with_bias:
        zrb_sb = nc.alloc_sbuf_tensor("zrb_sb", [128, 512], f32)
        bias_a_sb = nc.alloc_sbuf_tensor("bias_a_sb", [128, 512], f32)
        bias_h_sb = nc.alloc_sbuf_tensor("bias_h_sb", [64, 512], f32)

    # ---- PSUM (bank-disjoint readers/writers) ----
    psA = [nc.alloc_psum_tensor(f"psA{p}", [128, 512], f32) for p in (0, 1)]
    psH = [nc.alloc_psum_tensor(f"psH{p}", [64, 512], f32) for p in (0, 1)]  # hl
    psX = [nc.alloc_psum_tensor(f"psX{p}", [64, 512], f32) for p in (0, 1)]  # xh
    psT = [nc.alloc_psum_tensor(f"psT{p}", [128, 256], f32) for p in (0, 1)]

    # ---- semaphores ----
    init_sem = nc.alloc_semaphore("init_sem")
    mm_sem = nc.alloc_semaphore("mm_sem")    # +4/step: A, hl, xh, transposes
    act_sem = nc.alloc_semaphore("act_sem")  # +3/step: sigmoid, tanh, slot0 copy
    dve_sem = nc.alloc_semaphore("dve_sem")  # +3/step: tanh-in, h16, h32
    rsem1 = nc.alloc_semaphore("rsem1")      # partner S1 arrivals: +2/step
    bsem = nc.alloc_semaphore("bsem")        # local send complete: +16/step
    prep_sem = nc.alloc_semaphore("prep_sem")
    out_sem = nc.alloc_semaphore("out_sem")
    if with_bias:
        zb_sem = nc.alloc_semaphore("zb_sem")

    N_LOADS = (7 if with_bias else 5)

    def htile(p, tt):
        return ht_sb[p][:, tt * 64:(tt + 1) * 64]

    with nc.Block() as block:

        @block.sync
        def _(sync):
            sync.dma_start(out=wg_sb[:, :], in_=wg[:, :]).then_inc(init_sem, 16)
            sync.dma_start(out=u1_sb[:, :], in_=u1[:, :]).then_inc(init_sem, 16)
            sync.dma_start(out=ht_sb[0][:, :], in_=ht0[:, :]).then_inc(init_sem, 16)
            sync.dma_start(out=h32_sb[0][:, :], in_=x32[:, :]).then_inc(init_sem, 16)
            sync.dma_start(out=ident_sb[:, :], in_=ident[:, :]).then_inc(init_sem, 16)
            if with_bias:
                sync.dma_start(out=bias_a_sb[:, :], in_=bias_a[:, :]).then_inc(init_sem, 16)
                sync.dma_start(out=bias_h_sb[:, :], in_=bias_h[:, :]).then_inc(init_sem, 16)
            for t in range(t_steps):
                nxt = (t + 1) % 2
                sync.dma_start(out=out[t], in_=h32_sb[nxt][:, :]).then_inc(
                    out_sem, 16)._wait_ge(dve_sem, 4 * t + 4)

        @block.tensor
        def _(tensor):
            for t in range(t_steps):
                par = t % 2
                rhs_w, blk = (u1_sb, 1536) if t == 0 else (wg_sb, 2048)
                # passes: (psum-ap-maker, col-offset-in-block, start, inc)
                if t == 0:
                    # u1 layout [Uz | Ur | Uh]; no xh pass, hl tail incs +2
                    passes = [
                        (psA[par][0:64, :], 0, True, 0),        # z
                        (psA[par][64:128, :], 512, True, 1),    # r -> A done
                        (psH[par][:, :], 1024, True, 2),        # hl (+2: no xh)
                    ]
                else:
                    passes = [
                        (psA[par][0:64, :], 0, True, 0),        # z
                        (psA[par][64:128, :], 512, True, 1),    # r -> A done
                        (psH[par][:, :], 1536, True, 1),        # hl
                        (psX[par][:, :], 1024, True, 1),        # xh
                    ]
                # phase A: all own-tile (slot0) matmuls of every pass, so
                # the PE never stalls on the partner exchange until half the
                # step's matmuls are already issued; phase B (slot1 tiles)
                # carries the rsem1 gate once. Accumulation groups interleave
                # across banks (start on first phase-A touch, stop on last
                # phase-B touch).
                for phase, tts in ((0, range(0, KT // 2)),
                                   (1, range(KT // 2, KT))):
                    for pi, (ps_ap, co, start0, inc) in enumerate(passes):
                        for tt in tts:
                            mm = tensor.matmul(
                                ps_ap,
                                htile(par, tt),
                                rhs_w[:, tt * blk + co: tt * blk + co + 512],
                                start=(tt == 0 and start0),
                                stop=(tt == KT - 1),
                                skip_group_check=True)
                            if phase == 0 and pi == 0 and tt == 0:
                                if t == 0:
                                    mm._wait_ge(init_sem, 16 * N_LOADS)
                                else:
                                    # own slot0 tiles: ACT copy of step t-1
                                    mm._wait_ge(act_sem, 3 * t)
                            if phase == 1 and pi == 0 and tt == KT // 2 \
                                    and t > 0:
                                # partner slot1 tiles: S1 of step t-1 landed
                                mm._wait_ge(rsem1, 2 * t)
                        if phase == 1 and inc:
                            mm.then_inc(mm_sem, inc)
                # transposed h' built by PSUM accumulation: psT = u.T + v.T.
                # The u transposes run as soon as the sigmoid-side DVE ops
                # finish (mid-step); only v's transposes trail the tanh.
                for src_sb, dv, st in ((d_sb, 4 * t + 1, True),
                                       (t1_sb, 4 * t + 3, False)):
                    for i in range(4):
                        tr = tensor.matmul(
                            psT[par][:, i * 64:(i + 1) * 64],
                            src_sb[:, i * 128:(i + 1) * 128],
                            ident_sb[:, :],
                            is_transpose=True, start=(st and i == 0), stop=not st,
                            skip_group_check=True)
                        if i == 0:
                            tr._wait_ge(dve_sem, dv)
                tr.then_inc(mm_sem, 1)

        @block.scalar
        def _(scalar):
            for t in range(t_steps):
                par, nxt = t % 2, (t + 1) % 2
                if with_bias:
                    sig = scalar.activation(zr_sb[:, :], zrb_sb[:, :],
                                            Act.Sigmoid)._wait_ge(zb_sem, t + 1)
                else:
                    sig = scalar.activation(zr_sb[:, :], psA[par][:, :],
                                            Act.Sigmoid)._wait_ge(mm_sem, 4 * t + 1)
                sig.then_inc(act_sem, 1)
                tin = t1_sb if t == 0 else t2_sb
                scalar.activation(hh_sb[:, :], tin[:, :], Act.Tanh)._wait_ge(
                    dve_sem, 4 * t + 2).then_inc(act_sem, 1)
                scalar.copy(ht_sb[nxt][:, 0:256], psT[par][:, :])._wait_ge(
                    mm_sem, 4 * t + 4).then_inc(act_sem, 1)

        @block.vector
        def _(vector):
            for t in range(t_steps):
                par, nxt = t % 2, (t + 1) % 2
                if with_bias:
                    # zrb = psA + bias (sigmoid input); done on DVE since the
                    # ACT bias operand is per-partition and batch-major bias
                    # varies along the free dim. b==0 compiles this out.
                    vector.tensor_tensor(zrb_sb[:, :], psA[par][:, :],
                                         bias_a_sb[:, :], Alu.add)._wait_ge(
                        mm_sem, 4 * t + 1).then_inc(zb_sem, 1)
                # u/g need only the sigmoid; t1 = r * hl also needs the hl
                # pass (one wait per instruction -> standalone act wait)
                vector.wait_ge(act_sem, 3 * t + 1)
                # u = z*h and g = 1-z need only the sigmoid; emitted FIRST so
                # u's dve inc (4t+1) unblocks the early u-transposes while
                # the PE still streams hl/xh. h' = u.T + v.T accumulates in
                # PSUM, so no explicit add remains on the tail.
                vector.tensor_tensor(d_sb[:, :], zr_sb[0:64, :],
                                     h32_sb[par][:, :], Alu.mult).then_inc(
                    dve_sem, 1)                                    # u
                vector.tensor_scalar(m_sb[:, :], zr_sb[0:64, :], -1.0, 1.0,
                                     Alu.mult, Alu.add)            # g
                tt1 = vector.tensor_tensor(
                    t1_sb[:, :], zr_sb[64:128, :], psH[par][:, :],
                    Alu.mult)._wait_ge(mm_sem, 4 * t + 2)
                if t == 0:
                    last = tt1
                    if with_bias:
                        last = vector.tensor_tensor(t1_sb[:, :], t1_sb[:, :],
                                                    bias_h_sb[:, :], Alu.add)
                else:
                    last = vector.tensor_tensor(
                        t2_sb[:, :], t1_sb[:, :], psX[par][:, :],
                        Alu.add)._wait_ge(mm_sem, 4 * t + 3)
                    if with_bias:
                        last = vector.tensor_tensor(t2_sb[:, :], t2_sb[:, :],
                                                    bias_h_sb[:, :], Alu.add)
                last.then_inc(dve_sem, 1)
                vector.tensor_tensor(t1_sb[:, :], m_sb[:, :], hh_sb[:, :],
                                     Alu.mult)._wait_ge(
                    act_sem, 3 * t + 2).then_inc(dve_sem, 1)       # v
                if t >= 2:
                    vector.wait_ge(out_sem, 16 * (t - 1))
                vector.tensor_tensor(h32_sb[nxt][:, :], d_sb[:, :], t1_sb[:, :],
                                     Alu.add).then_inc(dve_sem, 1)

        @block.gpsimd
        def _(gpsimd):
            for t in range(t_steps - 1):
                nxt = (t + 1) % 2
                rdests = [None] * NCORES
                rdests[1] = (0, 1)
                gpsimd.remote_dma_broadcast(
                    ht_sb[nxt][:, 256:512],
                    ht_sb[nxt][:, 0:256],
                    remote_sem=rsem1,
                    local_sem=bsem,
                    rdests=rdests,
                ).then_inc(prep_sem, 1)
                gpsimd.trigger_dma(1)._wait_ge(act_sem, 3 * t + 3)

    nc.compile()
    return nc


# ---------------------------------------------------------------------------
# host side
# ---------------------------------------------------------------------------

def _prep_inputs(x, W, U, b):
    x = np.asarray(x, np.float32)
    W = np.asarray(W, np.float32)
    U = np.asarray(U, np.float32)
    b = np.asarray(b, np.float32)
    with_bias = bool(np.any(b != 0.0))

    Wz, Wr, Wh = W[:, :D], W[:, D:2 * D], W[:, 2 * D:]
    Uz, Ur, Uh = U[:, :D], U[:, D:2 * D], U[:, 2 * D:]
    G = [Wz + Uz, Wr + Ur, Wh, Uh]   # z | r | xh | hl
    U1 = [Uz, Ur, Uh]

    in_maps = []
    for c in range(NCORES):
        m, p = c & 1, c >> 1
        bsl = slice(p * BQ, (p + 1) * BQ)
        fsl = slice(m * FH, (m + 1) * FH)
        wblocks, ublocks, hblocks = [], [], []
        for j in (0, 1):             # slot: 0 = own half, 1 = partner half
            half = m ^ j
            for i in range(4):       # k-tile within the half
                rows = slice(half * FH + i * 128, half * FH + (i + 1) * 128)
                wblocks.append(np.hstack([g[rows, fsl] for g in G]))
                ublocks.append(np.hstack([g[rows, fsl] for g in U1]))
                hblocks.append(x[bsl, rows].T)
        in_map = {
            "wg": np.ascontiguousarray(np.hstack(wblocks)).astype(np.float16),
            "u1": np.ascontiguousarray(np.hstack(ublocks)).astype(np.float16),
            "ht0": np.ascontiguousarray(np.hstack(hblocks)).astype(np.float16),
            "x32": np.ascontiguousarray(x[bsl, fsl]),
            "ident": np.eye(64, dtype=np.float32),
        }
        if with_bias:
            bz, br, bh = b[:D][fsl], b[D:2 * D][fsl], b[2 * D:][fsl]
            in_map["bias_a"] = np.ascontiguousarray(
                np.vstack([np.tile(bz, (64, 1)), np.tile(br, (64, 1))])
            ).astype(np.float32)
            in_map["bias_h"] = np.ascontiguousarray(
                np.tile(bh, (64, 1))).astype(np.float32)
        in_maps.append(in_map)
    return in_maps, with_bias


def gather(results, t_steps=T):
    """results: per-core dicts with 'out' [t_steps, 64, 512] -> [B, T, D]."""
    full = np.empty((B, t_steps, D), np.float32)
    for c in range(NCORES):
        m, p = c & 1, c >> 1
        co = np.asarray(results[c]["out"]).reshape(t_steps, BQ, FH)
        full[p * BQ:(p + 1) * BQ, :, m * FH:(m + 1) * FH] = \
            np.transpose(co, (1, 0, 2))
    return full


def run(x, W, U, b, trace=False, t_steps=T, **spmd_kwargs):
    import sys
    if "/opt/trn_rl_repo" not in sys.path:
        sys.path.insert(0, "/opt/trn_rl_repo")
    from concourse.bass_utils import run_bass_kernel_spmd

    in_maps, with_bias = _prep_inputs(x, W, U, b)
    nc = _build(t_steps, with_bias)
    res = run_bass_kernel_spmd(nc, in_maps, core_ids=list(range(NCORES)),
                               trace=trace, **spmd_kwargs)
    full = np.empty((B, t_steps, D), np.float32)
    for c in range(NCORES):
        m, p = c & 1, c >> 1
        co = np.asarray(res.results[c]["out"])  # [T, 64, 512]
        full[p * BQ:(p + 1) * BQ, :, m * FH:(m + 1) * FH] = \
            np.transpose(co, (1, 0, 2))
    return full, res


def kernel(x, W, U, b):
    return run(x, W, U, b)[0]
